# revision 1
# baseline (speedup 1.0000x reference)
"""Trainium2 Bass kernel for nn_Conv2DExperimental (MVN-sampled 3x3 conv).

Computation (per the nn.Module):
  L    = tril(weight_L, -1) + softplus(diag(weight_L)) * I      # [O,I,D,D], D=9
  w    = weight_loc + einsum('oiab,oib->oia', L, eps_w)         # [O,I,3,3]
  b    = bias_loc + eps_b * softplus(bias_ro)                   # [O]
  out  = conv2d(x, w, SAME, NCHW) + b

Distribution: data-parallel over the batch dim of x (32 images -> 8 cores x 4),
with the weight sampling replicated on every core (it is tiny).

Per-core kernel:
  - sampling runs on VectorE/ScalarE with O on the partition dim; the 9 sampled
    64x64 tap matrices are PE-transposed into block-diagonal [128,128] lhsT
    tiles pairing two images per matmul (partitions = (image, channel)).
  - conv runs as 9 shift-matmuls per 2-output-row PSUM tile in float32r
    (fp32 data truncated to FP22 in the PE; 1 cycle/row at N>=256).
  - ScalarE evacuates PSUM with the bias add fused; DMA engines stream
    row-strips of x in and finished strips of out back to HBM.
"""

import sys
from contextlib import ExitStack

for _p in ("/opt/trn_rl_repo",):
    if _p not in sys.path:
        sys.path.insert(0, _p)

import numpy as np

import concourse.bass as bass
import concourse.bacc as bacc
import concourse.mybir as mybir
from concourse.tile import TileContext

F32 = mybir.dt.float32
F32R = mybir.dt.float32r
AF = mybir.ActivationFunctionType

N_CORES = 8
O = 64
I = 64
KK = 3
D = KK * KK  # 9


def build_nc(nb=4, hh=224, ww=224, rstrip=28, x_bufs=3, o_bufs=2, passes=1):
    """Build the per-core Bass program.

    nb: images per core (must be even: images are processed in pairs)
    hh, ww: spatial dims; rstrip: output rows per strip (must divide hh, even)
    """
    assert nb % 2 == 0 and hh % rstrip == 0 and rstrip % 2 == 0
    wpad = ww + 2
    nstrips = hh // rstrip
    ntiles = rstrip // 2  # psum tiles (2 output rows each) per strip

    nc = bacc.Bacc("TRN2", target_bir_lowering=False, debug=False)

    x_t = nc.dram_tensor("x", [nb, I, hh, ww], F32R, kind="ExternalInput").ap()
    wl_t = nc.dram_tensor("wL", [O, I * D * D], F32, kind="ExternalInput").ap()
    wloc_t = nc.dram_tensor("wloc", [O, I * D], F32, kind="ExternalInput").ap()
    epsw_t = nc.dram_tensor("epsw", [O, I * D], F32, kind="ExternalInput").ap()
    ident_t = nc.dram_tensor("ident", [O, O], F32, kind="ExternalInput").ap()
    bias3_t = nc.dram_tensor("bias3", [3, O], F32, kind="ExternalInput").ap()
    out_t = nc.dram_tensor("out", [nb, O, hh, ww], F32, kind="ExternalOutput").ap()

    with TileContext(nc) as tc, ExitStack() as stack:
        # ---------------- weight + bias sampling (one-time prologue) --------
        cp = stack.enter_context(tc.tile_pool(name="consts", bufs=1))
        wl = cp.tile([O, I * D * D], F32, name="wl", tag="wl")
        wloc = cp.tile([O, I * D], F32, name="wloc_s", tag="wloc_s")
        epsw = cp.tile([O, I * D], F32, name="epsw_s", tag="epsw_s")
        ident = cp.tile([O, O], F32, name="ident_s", tag="ident_s")
        b3 = cp.tile([O, 3], F32, name="b3", tag="b3")
        sp = cp.tile([O, I * D], F32, name="sp", tag="sp")
        tmp = cp.tile([O, I * D], F32, name="tmp", tag="tmp")
        wsamp = cp.tile([O, I * D], F32, name="wsamp", tag="wsamp")
        bias = cp.tile([128, 1], F32, name="bias", tag="bias")
        # 9 block-diagonal lhsT tiles, stored side by side: [128, 9*128]
        wts = cp.tile([128, D * 128], F32R, name="wts", tag="wts")
        sp_b = cp.tile([O, 1], F32, name="sp_b", tag="sp_b")

        b3p = cp.tile([3, O], F32, name="b3p", tag="b3p")
        nc.sync.dma_start(wl[:], wl_t[:])
        nc.scalar.dma_start(b3p[:], bias3_t[:])
        nc.scalar.dma_start(ident[:], ident_t[:])
        nc.scalar.dma_start(wloc[:], wloc_t[:])
        nc.scalar.dma_start(epsw[:], epsw_t[:])

        # PE warm-up feed: zero tiles via GpSimd (idle queue, no input deps)
        # so the warm-up matmuls can start within ~1us of kernel entry.
        identr = cp.tile([O, O], F32R, name="identr", tag="identr")
        junk = cp.tile([O, 256], F32R, name="junk", tag="junk")
        with tc.high_priority():
            nc.gpsimd.memset(identr[:].bitcast(F32), 0.0)
            nc.gpsimd.memset(junk[:].bitcast(F32), 0.0)
        nc.gpsimd.memset(wts[:].bitcast(F32), 0.0)

        # PE warm-up: the HAM clock gate needs ~3.4us of sustained matmul
        # activity to lift the PE from 1.2 to 2.4 GHz, and re-throttles after
        # ~3.4us idle. One long accumulation group (no inter-matmul
        # semaphores) bridges the PE from kernel entry to the transposes.
        with tc.tile_pool(name="wp", bufs=1, space="PSUM") as wp:
            warm = wp.tile([O, 256], F32, name="warm")
            n_warm = 90
            for k in range(n_warm):
                nc.tensor.matmul(
                    warm[:], identr[:], junk[:],
                    start=(k == 0), stop=(k == n_warm - 1),
                )

            # bias3 arrives as [3, 64]; transpose to [64, 3] on the PE (a
            # partition-major DMA of 64x3 elements costs ~17us in descriptors)
            bp_ps = wp.tile([O, 3], F32, name="bp_ps")
            with tc.high_priority():
                nc.tensor.matmul(
                    bp_ps[:], b3p[:], ident[0:3, 0:3], start=True, stop=True
                )
                nc.vector.tensor_copy(b3[:], bp_ps[:])

        # softplus of the per-(o,i) diagonals: wl free layout is (i, d=a*9+b);
        # diagonal entries sit at d = 10*a  ->  sp layout (i, a).
        # ACT order Exp,Exp,Ln,Ln avoids activation-table reload thrash
        # (each ACT_TABLE_LOAD costs ~1.3us). softplus(x) = ln(exp(x) + 1):
        # there is no Softplus LUT in this toolchain.
        diag_view = bass.AP(
            tensor=wl[:].tensor,
            offset=wl[:].offset,
            ap=[list(p) for p in wl[:].ap[:1]] + [[D * D, I], [D + 1, D]],
        )
        sp3 = sp[:].rearrange("o (i a) -> o i a", i=I)
        with tc.high_priority():
            nc.scalar.activation(sp_b[:], b3[:, 1:2], AF.Exp)
            nc.scalar.activation(sp3, diag_view, AF.Exp)
            nc.scalar.activation(sp[:], sp[:], AF.Ln, bias=1.0)
            nc.scalar.activation(sp_b[:], sp_b[:], AF.Ln, bias=1.0)

        # bias = bias_loc + eps_b * softplus(bias_ro)
        nc.vector.tensor_mul(sp_b[:], sp_b[:], b3[:, 2:3])
        nc.vector.tensor_add(bias[0:O, :], b3[:, 0:1], sp_b[:])
        nc.scalar.dma_start(bias[O:128, :], bias[0:O, :])

        # wsamp = wloc + softplus(diag) * eps  (the b == a term of L @ eps)
        nc.vector.tensor_mul(tmp[:], sp[:], epsw[:])
        nc.vector.tensor_add(wsamp[:], wloc[:], tmp[:])

        # += strict-lower part: for each b,
        #   wsamp[o,(i,a)] += wl[o,(i,a*9+b)] * eps[o,(i,b)]  for a in b+1..8.
        # Restricting each view to a > b IS the tril(-1) mask.
        for b in range(D - 1):
            na = D - 1 - b  # taps strictly below the diagonal
            wl_b = bass.AP(
                tensor=wl[:].tensor,
                offset=wl[:].offset + (b + 1) * D + b,
                ap=[list(p) for p in wl[:].ap[:1]] + [[D * D, I], [D, na]],
            )
            eps_b = bass.AP(
                tensor=epsw[:].tensor,
                offset=epsw[:].offset + b,
                ap=[list(p) for p in epsw[:].ap[:1]] + [[D, I], [0, na]],
            )
            tmp_b = bass.AP(
                tensor=tmp[:].tensor,
                offset=tmp[:].offset + b + 1,
                ap=[list(p) for p in tmp[:].ap[:1]] + [[D, I], [1, na]],
            )
            ws_b = bass.AP(
                tensor=wsamp[:].tensor,
                offset=wsamp[:].offset + b + 1,
                ap=[list(p) for p in wsamp[:].ap[:1]] + [[D, I], [1, na]],
            )
            nc.vector.tensor_tensor(tmp_b, wl_b, eps_b, mybir.AluOpType.mult)
            nc.vector.tensor_add(ws_b, ws_b, tmp_b)

        # build the 9 block-diagonal lhsT tiles:
        #   wts[:, a*128:(a+1)*128] = [[T_a, 0], [0, T_a]],  T_a[i,o] = wsamp[o, i*9+a]
        with tc.tile_pool(name="pt", bufs=1, space="PSUM") as ptp:
            # transpose the 9 taps, packed 5 + 4 into two PSUM banks, then
            # two strided copies into the lhsT tile (disjoint column ranges:
            # start=True only on the first write of each bank)
            ptA = ptp.tile([O, 5 * O], F32, name="ptA")
            ptB = ptp.tile([O, 4 * O], F32, name="ptB")
            for a in range(D):
                w_a = bass.AP(
                    tensor=wsamp[:].tensor,
                    offset=wsamp[:].offset + a,
                    ap=[list(p) for p in wsamp[:].ap[:1]] + [[D, I]],
                )
                dst_pt = ptA if a < 5 else ptB
                c = a if a < 5 else a - 5
                nc.tensor.matmul(
                    dst_pt[:, c * O : (c + 1) * O],
                    w_a,
                    ident[:],
                    is_transpose=True,
                    start=(c == 0),
                    stop=(c == (4 if a < 5 else 3)),
                    skip_group_check=True,
                )
            for pt_t, a0, na_t in ((ptA, 0, 5), (ptB, 5, 4)):
                dst = bass.AP(
                    tensor=wts[0:O].tensor,
                    offset=wts[0:O].offset + a0 * 128,
                    ap=[list(p) for p in wts[0:O].ap[:1]] + [[128, na_t], [1, O]],
                )
                nc.vector.tensor_copy(dst, pt_t[:].rearrange("p (a o) -> p a o", o=O))
        # partition-shifted copy of the diagonal blocks: [0:64, a*128:+64] ->
        # [64:128, a*128+64:+64]. Two DMAs (taps 0-4 / 5-8) so the first
        # conv matmuls only wait on the first transpose batch, not the whole
        # sampling chain.
        wts_lo = wts[0:O]
        wts_hi = wts[O:128]
        for a0, na_t in ((0, 5), (5, 4)):
            src = bass.AP(
                tensor=wts_lo.tensor,
                offset=wts_lo.offset + a0 * 128,
                ap=[list(p) for p in wts_lo.ap[:1]] + [[128, na_t], [1, O]],
            )
            dst = bass.AP(
                tensor=wts_hi.tensor,
                offset=wts_hi.offset + a0 * 128 + O,
                ap=[list(p) for p in wts_hi.ap[:1]] + [[128, na_t], [1, O]],
            )
            nc.scalar.dma_start(dst, src)

        # ---------------- convolution ---------------------------------------
        xp = stack.enter_context(tc.tile_pool(name="xstrip", bufs=x_bufs))
        op = stack.enter_context(tc.tile_pool(name="ostrip", bufs=o_bufs))
        pp = stack.enter_context(tc.tile_pool(name="acc", bufs=8, space="PSUM"))
        for _pass in range(passes):
            for pair in range(nb // 2):
                n0 = 2 * pair
                strips = [(s * rstrip, rstrip) for s in range(nstrips)]
                if pair == nb // 2 - 1 and _pass == passes - 1 and rstrip >= 8:
                    # Taper the final strips so the kernel does not end on a
                    # full-size store DMA the PE has to wait out.
                    h_last = strips.pop()[0]
                    r = rstrip
                    while r > 4:
                        r1 = (r // 2 + 1) & ~1
                        strips.append((h_last, r1))
                        h_last += r1
                        r -= r1
                    strips.append((h_last, r))
                for h0, rout in strips:
                    xs = xp.tile([128, rstrip + 2, wpad], F32R, name="xs")
                    # zero the left/right halo columns
                    halo = bass.AP(
                        tensor=xs[:].tensor,
                        offset=xs[:].offset,
                        ap=[list(p) for p in xs[:].ap[:1]]
                        + [[wpad, rout + 2], [ww + 1, 2]],
                    )
                    nc.gpsimd.memset(halo.bitcast(F32), 0.0)
                    # load input rows [h0-1, h0+rout], clipped to the image
                    r_lo = max(h0 - 1, 0)
                    r_hi = min(h0 + rout + 1, hh)
                    dst_r0 = r_lo - (h0 - 1)
                    if h0 == 0:
                        nc.gpsimd.memset(xs[:, 0:1, :].bitcast(F32), 0.0)
                    if h0 + rout == hh:
                        nc.gpsimd.memset(
                            xs[:, rout + 1 : rout + 2, :].bitcast(F32), 0.0
                        )
                    src = x_t[n0 : n0 + 2, :, r_lo:r_hi, :].rearrange(
                        "n i h w -> (n i) h w"
                    )
                    nc.sync.dma_start(
                        xs[:, dst_r0 : dst_r0 + (r_hi - r_lo), 1 : ww + 1], src
                    )

                    os_ = op.tile([128, rout, ww], F32, name="os_")
                    for j in range(rout // 2):
                        acc = pp.tile([128, 2, ww], F32, name="acc")
                        for tap in range(D):
                            dy, dx = tap // 3 - 1, tap % 3 - 1
                            rhs = bass.AP(
                                tensor=xs[:].tensor,
                                offset=xs[:].offset
                                + (2 * j + 1 + dy) * wpad
                                + 1
                                + dx,
                                ap=[list(p) for p in xs[:].ap[:1]]
                                + [[wpad, 2], [1, ww]],
                            )
                            nc.tensor.matmul(
                                acc[:],
                                wts[:, tap * 128 : (tap + 1) * 128],
                                rhs,
                                start=(tap == 0),
                                stop=(tap == D - 1),
                            )
                        nc.scalar.activation(
                            os_[:, 2 * j : 2 * j + 2, :],
                            acc[:],
                            AF.Identity,
                            bias=bias[:, 0:1],
                        )
                    dst = out_t[n0 : n0 + 2, :, h0 : h0 + rout, :].rearrange(
                        "n i h w -> (n i) h w"
                    )
                    nc.sync.dma_start(dst, os_[:])

    nc.compile()
    return nc


_CACHED_NC = None


def _host_inputs(x_shard, weight_loc, weight_L, bias_loc, bias_ro, eps_w, eps_b):
    return {
        "x": np.ascontiguousarray(x_shard, np.float32),
        "wL": np.ascontiguousarray(weight_L.reshape(O, I * D * D), np.float32),
        "wloc": np.ascontiguousarray(weight_loc.reshape(O, I * D), np.float32),
        "epsw": np.ascontiguousarray(eps_w.reshape(O, I * D), np.float32),
        "ident": np.eye(O, dtype=np.float32),
        "bias3": np.ascontiguousarray(
            np.stack([bias_loc, bias_ro, eps_b]).astype(np.float32)
        ),
    }


def kernel(x, weight_loc, weight_L, bias_loc, bias_ro, eps_w, eps_b):
    global _CACHED_NC
    from concourse.bass_utils import run_bass_kernel_spmd

    x = np.asarray(x, np.float32)
    nb = x.shape[0] // N_CORES
    if _CACHED_NC is None:
        _CACHED_NC = build_nc(nb=nb)
    nc = _CACHED_NC

    in_maps = [
        _host_inputs(
            x[c * nb : (c + 1) * nb],
            np.asarray(weight_loc),
            np.asarray(weight_L),
            np.asarray(bias_loc),
            np.asarray(bias_ro),
            np.asarray(eps_w),
            np.asarray(eps_b),
        )
        for c in range(N_CORES)
    ]
    res = run_bass_kernel_spmd(nc, in_maps, list(range(N_CORES)))
    return np.concatenate([res.results[c]["out"] for c in range(N_CORES)], axis=0)



# revision 6
# speedup vs baseline: 1.4720x; 1.4720x over previous
"""Trainium2 Bass kernel for nn_Conv2DExperimental (MVN-sampled 3x3 conv).

Computation (per the nn.Module):
  L    = tril(weight_L, -1) + softplus(diag(weight_L)) * I      # [O,I,D,D], D=9
  w    = weight_loc + einsum('oiab,oib->oia', L, eps_w)         # [O,I,3,3]
  b    = bias_loc + eps_b * softplus(bias_ro)                   # [O]
  out  = conv2d(x, w, SAME, NCHW) + b
  with O = I = 64 channels, x [32, 64, 224, 224].

Distribution: data-parallel over the batch dim of x (32 images -> 8 cores x 4),
with the weight sampling replicated on every core (it is tiny).

Per-core kernel (row-parity conv, 75% PE utilization):
  - x is host-packed bf16 into SBUF layout [128, 113, 228]: partitions
    (parity q, in-channel), where q=0 slot k holds image row 2k and q=1 slot k
    holds row 2k-1 (staggered), columns padded by 2 on both sides.  Halo
    rows/columns are pre-zeroed on the host, so the kernel does no memsets
    and every DMA is one contiguous 51 KB run per partition.
  - output psum tiles are [128 = (row-parity p, out-channel), 2 pairs x 224]:
    out row 2k+p.  Per tile, 6 matmuls (2 input row-groups x 3 column shifts)
    apply all 9 taps exactly once per output: lhsT tiles have 3 of 4
    64x64 quadrants live (vs 2 of 4 for the image-paired block-diagonal
    scheme) -> 1.5x less PE time.
  - sampling runs on ScalarE/VectorE in bf16; the 9 tap matrices are
    PE-transposed into both partition halves at once (stride-0 free dim
    duplication), then VectorE/ScalarE assemble the 6 lhsT tiles.
  - ScalarE evacuates PSUM with the bias add fused (bf16 out); output is
    stored in a packed [8-strip, 128, 14, 224] layout the host re-interleaves.
"""

import sys
from contextlib import ExitStack

for _p in ("/opt/trn_rl_repo",):
    if _p not in sys.path:
        sys.path.insert(0, _p)

import numpy as np

import concourse.bass as bass
import concourse.bacc as bacc
import concourse.mybir as mybir
from concourse.tile import TileContext

F32 = mybir.dt.float32
F32R = mybir.dt.float32r
BF16 = mybir.dt.bfloat16
AF = mybir.ActivationFunctionType

N_CORES = 8
O = 64
I = 64
KK = 3
D = KK * KK  # 9
HH = 224
WW = 224
NP = HH // 2 + 1  # 113 pair slots
WPAD = WW + 4  # 228: 2 zero cols each side (even f32 alignment)
GSTRIP = 8  # output strips per image
PPG = (HH // 2) // GSTRIP  # 14 output row-pairs per strip


def _t_src(ptA, ptB, t, half):
    """PSUM source block for transposed tap t, partition half (0|1)."""
    pt = ptA if t < 5 else ptB
    c0 = (t if t < 5 else t - 5) * O
    return pt[half * O : (half + 1) * O, c0 : c0 + O]


def build_nc(nb=4, n_warm1=72, n_warm2=16):
    """Build the per-core Bass program. nb: images per core."""
    nc = bacc.Bacc("TRN2", target_bir_lowering=False, debug=False)

    xp_t = nc.dram_tensor("xp", [nb, 128, NP, WPAD], BF16, kind="ExternalInput").ap()
    wl_t = nc.dram_tensor("wL", [O, I * D * D], BF16, kind="ExternalInput").ap()
    wloc_t = nc.dram_tensor("wloc", [O, I * D], BF16, kind="ExternalInput").ap()
    epsw_t = nc.dram_tensor("epsw", [O, I * D], BF16, kind="ExternalInput").ap()
    ident_t = nc.dram_tensor("ident", [O, O], F32, kind="ExternalInput").ap()
    identb_t = nc.dram_tensor("identb", [O, O], BF16, kind="ExternalInput").ap()
    bias3_t = nc.dram_tensor("bias3", [3, O], F32, kind="ExternalInput").ap()
    out_t = nc.dram_tensor(
        "out", [nb, GSTRIP, 128, PPG, WW], BF16, kind="ExternalOutput"
    ).ap()

    with TileContext(nc) as tc, ExitStack() as stack:
        # ---------------- weight + bias sampling (one-time prologue) --------
        cp = stack.enter_context(tc.tile_pool(name="consts", bufs=1))
        wl = cp.tile([O, I * D * D], BF16, name="wl", tag="wl")
        wloc = cp.tile([O, I * D], BF16, name="wloc_s", tag="wloc_s")
        epsw = cp.tile([O, I * D], BF16, name="epsw_s", tag="epsw_s")
        ident = cp.tile([O, O], F32, name="ident_s", tag="ident_s")
        identb = cp.tile([O, O], BF16, name="identb_s", tag="identb_s")
        b3 = cp.tile([O, 3], F32, name="b3", tag="b3")
        sp = cp.tile([O, I * D], BF16, name="sp", tag="sp")
        tmp = cp.tile([O, I * D], BF16, name="tmp", tag="tmp")
        low = cp.tile([O, I * D], BF16, name="low", tag="low")
        wsamp = cp.tile([O, I * D], BF16, name="wsamp", tag="wsamp")
        # sampled weights duplicated side by side: the tap transposes read
        # free dim (q, i) -> both partition halves of the [128, .] transpose
        # destination in one PE pass (no partition-shift DMA afterwards)
        wsampd = cp.tile([O, 2 * I * D], BF16, name="wsampd", tag="wsampd")
        bias = cp.tile([128, 1], F32, name="bias", tag="bias")
        # 6 lhsT tiles side by side: [128, 6*128] = A_s (s=0..2), B_s (3..5)
        wts = cp.tile([128, 6 * 128], BF16, name="wts", tag="wts")
        sp_b = cp.tile([O, 1], F32, name="sp_b", tag="sp_b")
        b3p = cp.tile([3, O], F32, name="b3p", tag="b3p")

        # x strips stream on the sync HWDGE ring; everything else on scalar.
        nc.scalar.dma_start(wl[:], wl_t[:])
        nc.scalar.dma_start(b3p[:], bias3_t[:])
        nc.scalar.dma_start(ident[:], ident_t[:])
        nc.scalar.dma_start(identb[:], identb_t[:])
        nc.scalar.dma_start(wloc[:], wloc_t[:])
        nc.scalar.dma_start(epsw[:], epsw_t[:])

        # PE warm-up feed: zero tiles via GpSimd (idle queue, no input deps).
        # Full 128-partition matmuls: 64-wide ones do NOT trip the HAM clock
        # gate (measured: 90x [64,256] warmup left the PE at 1.2 GHz).
        identr = cp.tile([128, 128], F32R, name="identr", tag="identr")
        junk = cp.tile([128, 448], F32R, name="junk", tag="junk")
        with tc.high_priority():
            nc.gpsimd.memset(identr[:].bitcast(F32), 0.0)
            nc.gpsimd.memset(junk[:].bitcast(F32), 0.0)
        # zero the dead lhsT quadrants (A: q1/p1, B: q0/p0) in one shot
        nc.gpsimd.memset(wts[:].bitcast(F32), 0.0)
        # strict-lower accumulator: col a=0 of each (i, a) block is never
        # written by the b-loop below
        nc.gpsimd.memset(low[:].bitcast(F32), 0.0)

        # PE warm-up: HAM needs ~3.4us of sustained full-width matmul
        # activity to lift the PE 1.2 -> 2.4 GHz. One long accumulation
        # group bridges kernel entry to the tap transposes.
        with tc.tile_pool(name="wp", bufs=1, space="PSUM") as wp:
            warm = wp.tile([128, 448], F32, name="warm")
            for k in range(n_warm1):
                nc.tensor.matmul(
                    warm[:], identr[:], junk[:],
                    start=(k == 0), stop=(k == n_warm1 - 1),
                )

            # bias3 arrives as [3, 64]; transpose to [64, 3] on the PE (a
            # partition-major DMA of 64x3 elements costs ~17us in descriptors)
            bp_ps = wp.tile([O, 3], F32, name="bp_ps")
            with tc.high_priority():
                nc.tensor.matmul(
                    bp_ps[:], b3p[:], ident[0:3, 0:3], start=True, stop=True
                )
                nc.vector.tensor_copy(b3[:], bp_ps[:])

        # softplus of the per-(o,i) diagonals: wl free layout is (i, d=a*9+b);
        # diagonal entries sit at d = 10*a  ->  sp layout (i, a).
        # ACT order Exp,Exp,Ln,Ln avoids activation-table reload thrash
        # (each ACT_TABLE_LOAD costs ~1.3us). softplus(x) = ln(exp(x) + 1):
        # there is no Softplus LUT in this toolchain.
        diag_view = bass.AP(
            tensor=wl[:].tensor,
            offset=wl[:].offset,
            ap=[list(p) for p in wl[:].ap[:1]] + [[D * D, I], [D + 1, D]],
        )
        sp3 = sp[:].rearrange("o (i a) -> o i a", i=I)
        with tc.high_priority():
            nc.scalar.activation(sp_b[:], b3[:, 1:2], AF.Exp)
            nc.scalar.activation(sp3, diag_view, AF.Exp)
            nc.scalar.activation(sp[:], sp[:], AF.Ln, bias=1.0)
            nc.scalar.activation(sp_b[:], sp_b[:], AF.Ln, bias=1.0)

        # bias = bias_loc + eps_b * softplus(bias_ro)
        nc.vector.tensor_mul(sp_b[:], sp_b[:], b3[:, 2:3])
        nc.vector.tensor_add(bias[0:O, :], b3[:, 0:1], sp_b[:])
        nc.scalar.dma_start(bias[O:128, :], bias[0:O, :])

        # strict-lower part of L @ eps, independent of the softplus chain:
        #   low[o,(i,a)] = sum_b wl[o,(i,a*9+b)] * eps[o,(i,b)]  for a > b.
        # Restricting each view to a > b IS the tril(-1) mask.
        for b in range(D - 1):
            na = D - 1 - b  # taps strictly below the diagonal
            wl_b = bass.AP(
                tensor=wl[:].tensor,
                offset=wl[:].offset + (b + 1) * D + b,
                ap=[list(p) for p in wl[:].ap[:1]] + [[D * D, I], [D, na]],
            )
            eps_b = bass.AP(
                tensor=epsw[:].tensor,
                offset=epsw[:].offset + b,
                ap=[list(p) for p in epsw[:].ap[:1]] + [[D, I], [0, na]],
            )
            tmp_b = bass.AP(
                tensor=tmp[:].tensor,
                offset=tmp[:].offset + b + 1,
                ap=[list(p) for p in tmp[:].ap[:1]] + [[D, I], [1, na]],
            )
            low_b = bass.AP(
                tensor=low[:].tensor,
                offset=low[:].offset + b + 1,
                ap=[list(p) for p in low[:].ap[:1]] + [[D, I], [1, na]],
            )
            nc.vector.tensor_tensor(tmp_b, wl_b, eps_b, mybir.AluOpType.mult)
            nc.vector.tensor_add(low_b, low_b, tmp_b)

        # wsamp = wloc + softplus(diag) * eps + low  (written twice, see wsampd)
        nc.vector.tensor_mul(tmp[:], sp[:], epsw[:])
        nc.vector.tensor_add(wsamp[:], wloc[:], tmp[:])
        nc.vector.tensor_add(wsampd[:, 0 : I * D], wsamp[:], low[:])
        nc.vector.tensor_add(wsampd[:, I * D : 2 * I * D], wsamp[:], low[:])

        # ---- build the 6 lhsT tiles ----------------------------------------
        # A_s (slot s, s = dx+1): out col c takes input col c+dx.
        #   [q0,p0] = T[3+s] (dy 0)   [q0,p1] = T[s]   (dy -1)
        #   [q1,p0] = T[s]   (dy -1)  [q1,p1] = 0
        # B_s (slot 3+s):
        #   [q0,p0] = 0               [q0,p1] = T[6+s] (dy +1)
        #   [q1,p0] = T[6+s] (dy +1)  [q1,p1] = T[3+s] (dy 0)
        # where T[t][ich, och] = wsamp[och, ich*9 + t].
        with tc.tile_pool(name="pt", bufs=1, space="PSUM") as ptp:
            # transpose the 9 taps, packed 5 + 4 into two PSUM banks. The
            # stride-0 free dim duplicates each tap into BOTH partition
            # halves of the [128, .] destination, so no partition-shift DMA
            # is needed afterwards.
            ptA = ptp.tile([128, 5 * O], BF16, name="ptA")
            ptB = ptp.tile([128, 4 * O], BF16, name="ptB")
            for a in range(D):
                w_a2 = bass.AP(
                    tensor=wsampd[:].tensor,
                    offset=wsampd[:].offset + a,
                    ap=[list(p) for p in wsampd[:].ap[:1]] + [[I * D, 2], [D, I]],
                )
                dst_pt = ptA if a < 5 else ptB
                c = a if a < 5 else a - 5
                nc.tensor.matmul(
                    dst_pt[:, c * O : (c + 1) * O],
                    w_a2,
                    identb[:],
                    is_transpose=True,
                    start=(c == 0),
                    stop=(c == (4 if a < 5 else 3)),
                    skip_group_check=True,
                )

            # keep the PE busy while VectorE/ScalarE assemble the lhsT tiles
            warm2 = ptp.tile([128, 448], F32, name="warm2")
            for k in range(n_warm2):
                nc.tensor.matmul(
                    warm2[:], identr[:], junk[:],
                    start=(k == 0), stop=(k == n_warm2 - 1),
                )

            copies = []
            for s in range(3):
                cA = s * 128
                cB = (3 + s) * 128
                copies += [
                    (wts[0:O, cA : cA + O], _t_src(ptA, ptB, 3 + s, 0)),
                    (wts[0:O, cA + O : cA + 128], _t_src(ptA, ptB, s, 0)),
                    (wts[O:128, cA : cA + O], _t_src(ptA, ptB, s, 1)),
                    (wts[0:O, cB + O : cB + 128], _t_src(ptA, ptB, 6 + s, 0)),
                    (wts[O:128, cB : cB + O], _t_src(ptA, ptB, 6 + s, 1)),
                    (wts[O:128, cB + O : cB + 128], _t_src(ptA, ptB, 3 + s, 1)),
                ]
            for i, (dst, src) in enumerate(copies):
                if i % 2 == 0:
                    nc.vector.tensor_copy(dst, src)
                else:
                    nc.scalar.activation(dst, src, AF.Copy)

        # ---------------- convolution ---------------------------------------
        # per psum tile t (out rows 4t..4t+3 of one image):
        #   acc[(p,och), (k in {2t,2t+1}, c)] = out row 2k+p
        #   A_s: rhs slots (2t, 2t+1)   B_s: rhs slots (2t+1, 2t+2)
        #   rhs col start = s+1 (packed col cc = image col + 2)
        xp = stack.enter_context(tc.tile_pool(name="ximg", bufs=2))
        op = stack.enter_context(tc.tile_pool(name="ostrip", bufs=2))
        pp = stack.enter_context(tc.tile_pool(name="acc", bufs=8, space="PSUM"))
        half = NP // 2  # 56: tiles 0..27 need slots <= 56
        for n in range(nb):
            xs = xp.tile([128, NP, WPAD], BF16, name="xs")
            nc.sync.dma_start(xs[:, 0 : half + 1, :], xp_t[n, :, 0 : half + 1, :])
            nc.sync.dma_start(xs[:, half + 1 :, :], xp_t[n, :, half + 1 :, :])
            for g in range(GSTRIP):
                os_ = op.tile([128, PPG, WW], BF16, name="os_")
                last = n == nb - 1 and g == GSTRIP - 1
                for tt in range(PPG // 2):
                    t = (PPG // 2) * g + tt
                    acc = pp.tile([128, 2, WW], F32, name="acc")
                    for s in range(3):
                        rhs_a = xs[:, 2 * t : 2 * t + 2, s + 1 : s + 1 + WW]
                        rhs_b = xs[:, 2 * t + 1 : 2 * t + 3, s + 1 : s + 1 + WW]
                        nc.tensor.matmul(
                            acc[:],
                            wts[:, s * 128 : (s + 1) * 128],
                            rhs_a,
                            start=(s == 0),
                            stop=False,
                        )
                        nc.tensor.matmul(
                            acc[:],
                            wts[:, (3 + s) * 128 : (4 + s) * 128],
                            rhs_b,
                            start=False,
                            stop=(s == 2),
                        )
                    nc.scalar.activation(
                        os_[:, 2 * tt : 2 * tt + 2, :],
                        acc[:],
                        AF.Identity,
                        bias=bias[:, 0:1],
                    )
                    if last and tt == 3:
                        # taper: stream out the first half of the final strip
                        # so the kernel does not end on a full-size store
                        nc.scalar.dma_start(
                            out_t[n, g, :, 0:8, :], os_[:, 0:8, :]
                        )
                if last:
                    nc.scalar.dma_start(out_t[n, g, :, 8:, :], os_[:, 8:, :])
                else:
                    nc.scalar.dma_start(out_t[n, g], os_[:])

    nc.compile()
    return nc


_CACHED_NC = None


def _pack_x(x_shard_bf):
    """[nb, 64, 224, 224] bf16 -> [nb, 128, 113, 228] staggered parity pack."""
    nb = x_shard_bf.shape[0]
    xp = np.zeros((nb, 128, NP, WPAD), dtype=x_shard_bf.dtype)
    xp[:, 0:64, 0 : HH // 2, 2 : WW + 2] = x_shard_bf[:, :, 0::2, :]
    xp[:, 64:128, 1 : HH // 2 + 1, 2 : WW + 2] = x_shard_bf[:, :, 1::2, :]
    return xp


def _host_inputs(x_shard, weight_loc, weight_L, bias_loc, bias_ro, eps_w, eps_b):
    import ml_dtypes

    bf = ml_dtypes.bfloat16
    return {
        "xp": _pack_x(np.asarray(x_shard).astype(bf)),
        "wL": np.ascontiguousarray(weight_L.reshape(O, I * D * D)).astype(bf),
        "wloc": np.ascontiguousarray(weight_loc.reshape(O, I * D)).astype(bf),
        "epsw": np.ascontiguousarray(eps_w.reshape(O, I * D)).astype(bf),
        "ident": np.eye(O, dtype=np.float32),
        "identb": np.eye(O, dtype=np.float32).astype(bf),
        "bias3": np.ascontiguousarray(
            np.stack([bias_loc, bias_ro, eps_b]).astype(np.float32)
        ),
    }


def kernel(x, weight_loc, weight_L, bias_loc, bias_ro, eps_w, eps_b):
    global _CACHED_NC
    from concourse.bass_utils import run_bass_kernel_spmd

    x = np.asarray(x, np.float32)
    nb = x.shape[0] // N_CORES
    if _CACHED_NC is None:
        _CACHED_NC = build_nc(nb=nb)
    nc = _CACHED_NC

    import ml_dtypes

    x_bf = x.astype(ml_dtypes.bfloat16)
    in_maps = [
        _host_inputs(
            x_bf[c * nb : (c + 1) * nb],
            np.asarray(weight_loc),
            np.asarray(weight_L),
            np.asarray(bias_loc),
            np.asarray(bias_ro),
            np.asarray(eps_w),
            np.asarray(eps_b),
        )
        for c in range(N_CORES)
    ]
    res = run_bass_kernel_spmd(nc, in_maps, list(range(N_CORES)))
    outs = []
    for c in range(N_CORES):
        o = np.asarray(res.results[c]["out"])  # [nb, 8, 128, 14, 224] bf16
        o = o.reshape(nb, GSTRIP, 2, O, PPG, WW).transpose(0, 3, 1, 4, 2, 5)
        outs.append(o.reshape(nb, O, HH, WW).astype(np.float32))
    return np.concatenate(outs, axis=0)


# revision 8
# speedup vs baseline: 1.5586x; 1.0588x over previous
"""Trainium2 Bass kernel for nn_Conv2DExperimental (MVN-sampled 3x3 conv).

Computation (per the nn.Module):
  L    = tril(weight_L, -1) + softplus(diag(weight_L)) * I      # [O,I,D,D], D=9
  w    = weight_loc + einsum('oiab,oib->oia', L, eps_w)         # [O,I,3,3]
  b    = bias_loc + eps_b * softplus(bias_ro)                   # [O]
  out  = conv2d(x, w, SAME, NCHW) + b
  with O = I = 64 channels, x [32, 64, 224, 224].

Distribution: data-parallel over the batch dim of x (32 images -> 8 cores x 4),
with the weight sampling replicated on every core (it is tiny).

Per-core kernel (row-parity conv, 75% PE utilization):
  - x is host-packed bf16 into SBUF layout [128, 113, 228]: partitions
    (parity q, in-channel), where q=0 slot k holds image row 2k and q=1 slot k
    holds row 2k-1 (staggered), columns padded by 2 on both sides.  Halo
    rows/columns are pre-zeroed on the host, so the kernel needs no memsets
    and every x DMA is one contiguous ~51KB run per partition.
  - output psum tiles are [128 = (row-parity p, out-channel), 2 pairs x 224]:
    out row 2k+p.  Per tile, 6 matmuls (2 input row-groups x 3 column shifts)
    apply all 9 taps exactly once per output: lhsT tiles have 3 of 4
    64x64 quadrants live (vs 2 of 4 for the image-paired block-diagonal
    scheme) -> 1.5x less PE time.
  - sampling: the host pre-layouts weight_L as a masked strict-lower
    [O, (b, i, a)] block plus the diagonal [O, (i, a)] (pure data reshuffle),
    so L @ eps is one contiguous VectorE multiply + 3 tree adds, and
    softplus(diag) is a contiguous ScalarE op.  The 9 tap matrices are
    PE-transposed into both partition halves at once (weights duplicated
    side by side), then 8 batched strided copies assemble the 6 lhsT tiles.
  - ScalarE evacuates PSUM with the bias add fused (bf16 out); output is
    stored in a packed [8-strip, 128, 14, 224] layout the host re-interleaves.
"""

import sys
from contextlib import ExitStack

for _p in ("/opt/trn_rl_repo",):
    if _p not in sys.path:
        sys.path.insert(0, _p)

import numpy as np

import concourse.bass as bass
import concourse.bacc as bacc
import concourse.mybir as mybir
from concourse.tile import TileContext

F32 = mybir.dt.float32
F32R = mybir.dt.float32r
BF16 = mybir.dt.bfloat16
AF = mybir.ActivationFunctionType

N_CORES = 8
O = 64
I = 64
KK = 3
D = KK * KK  # 9
ID = I * D  # 576
HH = 224
WW = 224
NP = HH // 2 + 1  # 113 pair slots
WPAD = WW + 4  # 228: 2 zero cols each side
GSTRIP = 8  # output strips per image
PPG = (HH // 2) // GSTRIP  # 14 output row-pairs per strip


def build_nc(nb=4, n_w1=10, n_w1b=5, n_w2=9):
    """Build the per-core Bass program. nb: images per core."""
    nc = bacc.Bacc("TRN2", target_bir_lowering=False, debug=False)

    xp_t = nc.dram_tensor("xp", [nb, 128, NP, WPAD], BF16, kind="ExternalInput").ap()
    wl2_t = nc.dram_tensor("wl2", [O, 8 * ID], BF16, kind="ExternalInput").ap()
    diag_t = nc.dram_tensor("diag", [O, ID], BF16, kind="ExternalInput").ap()
    epsw_t = nc.dram_tensor("epsw", [O, ID], BF16, kind="ExternalInput").ap()
    eps2_t = nc.dram_tensor("eps2", [O, 8 * I], BF16, kind="ExternalInput").ap()
    wloc_t = nc.dram_tensor("wloc", [O, ID], BF16, kind="ExternalInput").ap()
    ident_t = nc.dram_tensor("ident", [O, O], F32, kind="ExternalInput").ap()
    identb_t = nc.dram_tensor("identb", [O, O], BF16, kind="ExternalInput").ap()
    bias3_t = nc.dram_tensor("bias3", [3, O], F32, kind="ExternalInput").ap()
    out_t = nc.dram_tensor(
        "out", [nb, GSTRIP, 128, PPG, WW], BF16, kind="ExternalOutput"
    ).ap()

    with TileContext(nc) as tc, ExitStack() as stack:
        # ---------------- weight + bias sampling (one-time prologue) --------
        cp = stack.enter_context(tc.tile_pool(name="consts", bufs=1))
        wl2 = cp.tile([O, 8 * ID], BF16, name="wl2", tag="wl2")
        diag = cp.tile([O, ID], BF16, name="diag", tag="diag")
        epsw = cp.tile([O, ID], BF16, name="epsw_s", tag="epsw_s")
        eps2 = cp.tile([O, 8 * I], BF16, name="eps2", tag="eps2")
        wloc = cp.tile([O, ID], BF16, name="wloc_s", tag="wloc_s")
        ident = cp.tile([O, O], F32, name="ident_s", tag="ident_s")
        identb = cp.tile([O, O], BF16, name="identb_s", tag="identb_s")
        b3 = cp.tile([O, 3], F32, name="b3", tag="b3")
        b3p = cp.tile([3, O], F32, name="b3p", tag="b3p")
        sp = cp.tile([O, ID], BF16, name="sp", tag="sp")
        tmp = cp.tile([O, ID], BF16, name="tmp", tag="tmp")
        prod = cp.tile([O, 8 * ID], BF16, name="prod", tag="prod")
        wsamp = cp.tile([O, ID], BF16, name="wsamp", tag="wsamp")
        # sampled weights duplicated side by side: the tap transposes read
        # free dim (q, i) -> both partition halves of the [128, .] transpose
        # destination in one PE pass (no partition-shift DMA afterwards)
        wsampd = cp.tile([O, 2 * ID], BF16, name="wsampd", tag="wsampd")
        bias = cp.tile([128, 1], F32, name="bias", tag="bias")
        sp_b = cp.tile([O, 1], F32, name="sp_b", tag="sp_b")
        # 6 lhsT tiles side by side: [128, 6*128] = A_s (s=0..2), B_s (3..5)
        wts = cp.tile([128, 6 * 128], BF16, name="wts", tag="wts")

        # sampling inputs on the scalar HWDGE ring (x streams on sync; the
        # SDMA engines round-robin between the two rings at packet level,
        # so these small loads land within ~4us regardless of x traffic)
        nc.scalar.dma_start(wl2[:], wl2_t[:])
        nc.scalar.dma_start(diag[:], diag_t[:])
        nc.scalar.dma_start(eps2[:], eps2_t[:])
        nc.scalar.dma_start(epsw[:], epsw_t[:])
        nc.scalar.dma_start(wloc[:], wloc_t[:])
        nc.scalar.dma_start(identb[:], identb_t[:])
        nc.scalar.dma_start(ident[:], ident_t[:])
        nc.scalar.dma_start(b3p[:], bias3_t[:])

        # PE warm-up feed: zero tiles via VectorE (fast, no SWDGE latency).
        # Full 128-partition matmuls: 64-wide ones do NOT trip the HAM clock
        # gate (measured: 90x [64,256] warmup left the PE at 1.2 GHz).
        identr = cp.tile([128, 128], F32R, name="identr", tag="identr")
        junk = cp.tile([128, 448], F32R, name="junk", tag="junk")
        with tc.high_priority():
            nc.vector.memset(identr[:].bitcast(F32), 0.0)
            nc.vector.memset(junk[:].bitcast(F32), 0.0)
        # zero the dead lhsT quadrants (A: q1/p1, B: q0/p0) in one shot
        nc.gpsimd.memset(wts[:].bitcast(F32), 0.0)

        with tc.tile_pool(name="prol", bufs=1, space="PSUM") as wp:
            # HAM needs ~3.4us of sustained full-width matmul activity to
            # lift the PE 1.2 -> 2.4 GHz; these also bridge PE-idle windows
            # while VectorE/ScalarE run the sampling chain.
            warm = wp.tile([128, 448], F32, name="warm")
            for k in range(n_w1):
                nc.tensor.matmul(
                    warm[:], identr[:], junk[:], start=(k == 0), stop=(k == n_w1 - 1)
                )

            # bias3 arrives as [3, 64]; transpose to [64, 3] on the PE (a
            # partition-major DMA of 64x3 elements costs ~17us in descriptors)
            bp_ps = wp.tile([O, 3], F32, name="bp_ps")
            nc.tensor.matmul(bp_ps[:], b3p[:], ident[0:3, 0:3], start=True, stop=True)

            for k in range(n_w1b):
                nc.tensor.matmul(
                    warm[:], identr[:], junk[:], start=(k == 0), stop=(k == n_w1b - 1)
                )

            # ---- VectorE sampling chain (all contiguous bf16) ------------
            # prod[o,(b,i,a)] = wl2[o,(b,i,a)] * eps_w[o,(i,b)]; wl2 is
            # host-masked to the strict-lower taps, so summing b IS L@eps.
            eps2b = bass.AP(
                tensor=eps2[:].tensor,
                offset=eps2[:].offset,
                ap=[list(p) for p in eps2[:].ap[:1]] + [[I, 8], [1, I], [0, D]],
            )
            prod3 = prod[:].rearrange("o (b i a) -> o b i a", b=8, i=I)
            wl23 = wl2[:].rearrange("o (b i a) -> o b i a", b=8, i=I)
            nc.vector.tensor_tensor(prod3, wl23, eps2b, mybir.AluOpType.mult)
            nc.vector.tensor_add(prod[:, 0 : 4 * ID], prod[:, 0 : 4 * ID],
                                 prod[:, 4 * ID : 8 * ID])
            nc.vector.tensor_add(prod[:, 0 : 2 * ID], prod[:, 0 : 2 * ID],
                                 prod[:, 2 * ID : 4 * ID])
            nc.vector.tensor_add(prod[:, 0:ID], prod[:, 0:ID], prod[:, ID : 2 * ID])
            nc.vector.tensor_copy(b3[:], bp_ps[:])

            # softplus(diag) on ScalarE: Exp then Ln (ln(e^x + 1)); there is
            # no Softplus LUT in this toolchain.  Each table switch costs
            # ~1.3us, so the bias softplus (needs the PE bias transpose)
            # runs after, reusing nothing -- it is off the critical path.
            nc.scalar.activation(sp[:], diag[:], AF.Exp)
            nc.scalar.activation(sp[:], sp[:], AF.Ln, bias=1.0)

            # wsamp = wloc + softplus(diag)*eps + strict_lower (twice, for
            # the both-halves transpose trick)
            nc.vector.tensor_mul(tmp[:], sp[:], epsw[:])
            nc.vector.tensor_add(wsamp[:], wloc[:], tmp[:])
            nc.vector.tensor_add(wsampd[:, 0:ID], wsamp[:], prod[:, 0:ID])
            nc.vector.tensor_add(wsampd[:, ID : 2 * ID], wsamp[:], prod[:, 0:ID])

            # ---- tap transposes + lhsT assembly --------------------------
            # T[t][ich,och] = wsamp[och, ich*9+t], written to BOTH partition
            # halves of ptA/ptB at once via the duplicated wsampd free dim.
            ptA = wp.tile([128, 5 * O], BF16, name="ptA")
            ptB = wp.tile([128, 4 * O], BF16, name="ptB")
            for a in range(D):
                w_a2 = bass.AP(
                    tensor=wsampd[:].tensor,
                    offset=wsampd[:].offset + a,
                    ap=[list(p) for p in wsampd[:].ap[:1]] + [[ID, 2], [D, I]],
                )
                dst_pt = ptA if a < 5 else ptB
                c = a if a < 5 else a - 5
                nc.tensor.matmul(
                    dst_pt[:, c * O : (c + 1) * O],
                    w_a2,
                    identb[:],
                    is_transpose=True,
                    start=(c == 0),
                    stop=(c == (4 if a < 5 else 3)),
                    skip_group_check=True,
                )

            # keep the PE busy while the lhsT copies run
            for k in range(n_w2):
                nc.tensor.matmul(
                    warm[:], identr[:], junk[:], start=(k == 0), stop=(k == n_w2 - 1)
                )

            # batched strided copies (dst stride 128, src stride 64):
            #   A_s: [q0,p0]=T[3+s]  [q0,p1]=T[s]  [q1,p0]=T[s]  [q1,p1]=0
            #   B_s: [q0,p0]=0  [q0,p1]=T[6+s]  [q1,p0]=T[6+s]  [q1,p1]=T[3+s]
            def bcopy(eng, dst_c0, dst_n, src_pt, src_half, src_c0, ddst=128):
                dst = bass.AP(
                    tensor=wts[:].tensor,
                    offset=wts[:].offset + src_half * 64 * wts[:].ap[0][0] + dst_c0,
                    ap=[[wts[:].ap[0][0], 64], [ddst, dst_n], [1, O]],
                )
                src = src_pt[src_half * 64 : src_half * 64 + 64,
                             src_c0 : src_c0 + dst_n * O]
                src = bass.AP(
                    tensor=src.tensor, offset=src.offset,
                    ap=[list(src.ap[0])] + [[O, dst_n], [1, O]],
                )
                if eng == "v":
                    nc.vector.tensor_copy(dst, src)
                else:
                    nc.scalar.activation(dst, src, AF.Copy)

            bcopy("v", O, 3, ptA, 0, 0)        # A q0,p1 <- T[0..2]
            bcopy("v", 0, 3, ptA, 1, 0)        # A q1,p0 <- T[0..2]
            bcopy("v", 0, 2, ptA, 0, 3 * O)    # A q0,p0 <- T[3..4]
            bcopy("v", 2 * 128, 1, ptB, 0, 0)  # A2 q0,p0 <- T[5]
            bcopy("s", 3 * 128 + O, 3, ptB, 0, O)      # B q0,p1 <- T[6..8]
            bcopy("s", 3 * 128, 3, ptB, 1, O)          # B q1,p0 <- T[6..8]
            bcopy("s", 3 * 128 + O, 2, ptA, 1, 3 * O)  # B0-1 q1,p1 <- T[3..4]
            bcopy("s", 5 * 128 + O, 1, ptB, 1, 0)      # B2 q1,p1 <- T[5]

            # bias = bias_loc + eps_b * softplus(bias_ro)  (off critical path)
            nc.scalar.activation(sp_b[:], b3[:, 1:2], AF.Exp)
            nc.scalar.activation(sp_b[:], sp_b[:], AF.Ln, bias=1.0)
            nc.vector.tensor_mul(sp_b[:], sp_b[:], b3[:, 2:3])
            nc.vector.tensor_add(bias[0:O, :], b3[:, 0:1], sp_b[:])
            nc.scalar.dma_start(bias[O:128, :], bias[0:O, :])

        # ---------------- convolution ---------------------------------------
        # per psum tile t (out rows 4t..4t+3 of one image):
        #   acc[(p,och), (k in {2t,2t+1}, c)] = out row 2k+p
        #   A_s: rhs slots (2t, 2t+1)   B_s: rhs slots (2t+1, 2t+2)
        #   rhs col start = s+1 (packed col cc = image col + 2)
        xp = stack.enter_context(tc.tile_pool(name="ximg", bufs=2))
        op = stack.enter_context(tc.tile_pool(name="ostrip", bufs=2))
        pp = stack.enter_context(tc.tile_pool(name="acc", bufs=8, space="PSUM"))
        for n in range(nb):
            xs = xp.tile([128, NP, WPAD], BF16, name="xs")
            # image 0 in quarters so the first conv matmuls start ~8us in;
            # later images in halves (loads hide behind a full image of
            # compute).  Slot cut points: tile t reads slots 2t..2t+2.
            cuts = [0, 29, 57, 85, NP] if n == 0 else [0, 57, NP]
            for c0, c1 in zip(cuts[:-1], cuts[1:]):
                nc.sync.dma_start(xs[:, c0:c1, :], xp_t[n, :, c0:c1, :])
            for g in range(GSTRIP):
                os_ = op.tile([128, PPG, WW], BF16, name="os_")
                last = n == nb - 1 and g == GSTRIP - 1
                for tt in range(PPG // 2):
                    t = (PPG // 2) * g + tt
                    acc = pp.tile([128, 2, WW], F32, name="acc")
                    for s in range(3):
                        rhs_a = xs[:, 2 * t : 2 * t + 2, s + 1 : s + 1 + WW]
                        rhs_b = xs[:, 2 * t + 1 : 2 * t + 3, s + 1 : s + 1 + WW]
                        nc.tensor.matmul(
                            acc[:],
                            wts[:, s * 128 : (s + 1) * 128],
                            rhs_a,
                            start=(s == 0),
                            stop=False,
                        )
                        nc.tensor.matmul(
                            acc[:],
                            wts[:, (3 + s) * 128 : (4 + s) * 128],
                            rhs_b,
                            start=False,
                            stop=(s == 2),
                        )
                    nc.scalar.activation(
                        os_[:, 2 * tt : 2 * tt + 2, :],
                        acc[:],
                        AF.Identity,
                        bias=bias[:, 0:1],
                    )
                    # taper: stream the final strip out in pieces so the
                    # kernel does not end on a full-size store
                    if last and tt in (1, 3, 5):
                        k0, k1 = {1: (0, 4), 3: (4, 8), 5: (8, 12)}[tt]
                        nc.scalar.dma_start(
                            out_t[n, g, :, k0:k1, :], os_[:, k0:k1, :]
                        )
                if last:
                    nc.scalar.dma_start(out_t[n, g, :, 12:PPG, :], os_[:, 12:PPG, :])
                else:
                    nc.scalar.dma_start(out_t[n, g], os_[:])

    nc.compile()
    return nc


_CACHED_NC = None


def _pack_x(x_shard_bf):
    """[nb, 64, 224, 224] bf16 -> [nb, 128, 113, 228] staggered parity pack."""
    nb = x_shard_bf.shape[0]
    xp = np.zeros((nb, 128, NP, WPAD), dtype=x_shard_bf.dtype)
    xp[:, 0:64, 0 : HH // 2, 2 : WW + 2] = x_shard_bf[:, :, 0::2, :]
    xp[:, 64:128, 1 : HH // 2 + 1, 2 : WW + 2] = x_shard_bf[:, :, 1::2, :]
    return xp


def _host_inputs(x_shard, weight_loc, weight_L, bias_loc, bias_ro, eps_w, eps_b):
    import ml_dtypes

    bf = ml_dtypes.bfloat16
    wlf = np.asarray(weight_L, np.float32)  # [O, I, D(a), D(b)]
    mask = np.tril(np.ones((D, D), np.float32), -1)  # [a, b]: a > b
    wl2 = (wlf * mask).transpose(0, 3, 1, 2)[:, 0:8]  # [O, b, I, a]
    dg = np.diagonal(wlf, axis1=2, axis2=3)  # [O, I, D]
    ew = np.asarray(eps_w, np.float32)  # [O, I, D(b)]
    return {
        "xp": _pack_x(np.asarray(x_shard).astype(bf)),
        "wl2": np.ascontiguousarray(wl2.reshape(O, 8 * ID)).astype(bf),
        "diag": np.ascontiguousarray(dg.reshape(O, ID)).astype(bf),
        "epsw": np.ascontiguousarray(ew.reshape(O, ID)).astype(bf),
        "eps2": np.ascontiguousarray(
            ew.transpose(0, 2, 1)[:, 0:8].reshape(O, 8 * I)
        ).astype(bf),
        "wloc": np.ascontiguousarray(
            np.asarray(weight_loc, np.float32).reshape(O, ID)
        ).astype(bf),
        "ident": np.eye(O, dtype=np.float32),
        "identb": np.eye(O, dtype=np.float32).astype(bf),
        "bias3": np.ascontiguousarray(
            np.stack([bias_loc, bias_ro, eps_b]).astype(np.float32)
        ),
    }


def kernel(x, weight_loc, weight_L, bias_loc, bias_ro, eps_w, eps_b):
    global _CACHED_NC
    from concourse.bass_utils import run_bass_kernel_spmd

    x = np.asarray(x, np.float32)
    nb = x.shape[0] // N_CORES
    if _CACHED_NC is None:
        _CACHED_NC = build_nc(nb=nb)
    nc = _CACHED_NC

    import ml_dtypes

    x_bf = x.astype(ml_dtypes.bfloat16)
    in_maps = [
        _host_inputs(
            x_bf[c * nb : (c + 1) * nb],
            np.asarray(weight_loc),
            np.asarray(weight_L),
            np.asarray(bias_loc),
            np.asarray(bias_ro),
            np.asarray(eps_w),
            np.asarray(eps_b),
        )
        for c in range(N_CORES)
    ]
    res = run_bass_kernel_spmd(nc, in_maps, list(range(N_CORES)))
    outs = []
    for c in range(N_CORES):
        o = np.asarray(res.results[c]["out"])  # [nb, 8, 128, 14, 224] bf16
        o = o.reshape(nb, GSTRIP, 2, O, PPG, WW).transpose(0, 3, 1, 4, 2, 5)
        outs.append(o.reshape(nb, O, HH, WW).astype(np.float32))
    return np.concatenate(outs, axis=0)


# revision 10
# speedup vs baseline: 1.6100x; 1.0330x over previous
"""Trainium2 Bass kernel for nn_Conv2DExperimental (MVN-sampled 3x3 conv).

Computation (per the nn.Module):
  L    = tril(weight_L, -1) + softplus(diag(weight_L)) * I      # [O,I,D,D], D=9
  w    = weight_loc + einsum('oiab,oib->oia', L, eps_w)         # [O,I,3,3]
  b    = bias_loc + eps_b * softplus(bias_ro)                   # [O]
  out  = conv2d(x, w, SAME, NCHW) + b
  with O = I = 64 channels, x [32, 64, 224, 224].

Distribution: data-parallel over the batch dim of x (32 images -> 8 cores x 4),
with the weight sampling replicated on every core (it is tiny).

Per-core kernel (row-parity conv, 75% PE utilization):
  - x is host-packed bf16 into SBUF layout [128, 113, 228]: partitions
    (parity q, in-channel), where q=0 slot k holds image row 2k and q=1 slot k
    holds row 2k-1 (staggered), columns padded by 2 on both sides.  Halo
    rows/columns are pre-zeroed on the host, so the kernel needs no memsets.
    Images stream in as 29-slot quarters through a 6-deep tile pool, so the
    HBM reads are consumption-paced instead of bursting against the output
    stores (HBM is ~358 GB/s per core).
  - output psum tiles are [128 = (row-parity p, out-channel), 2 pairs x 224]:
    out row 2k+p.  Per tile, 6 matmuls (2 input row-groups x 3 column shifts)
    apply all 9 taps exactly once per output: lhsT tiles have 3 of 4
    64x64 quadrants live (vs 2 of 4 for the image-paired block-diagonal
    scheme) -> 1.5x less PE time.
  - sampling: the host pre-layouts weight_L as a masked strict-lower
    [O, (b, i, a)] block plus the diagonal [O, (i, a)], and pre-broadcasts
    eps_w to the same shape (pure data reshuffle), so L @ eps is one
    contiguous VectorE multiply + 3 tree adds, and softplus(diag) is a
    contiguous ScalarE op.  The 9 tap matrices are PE-transposed into both
    partition halves at once (weights duplicated side by side), then 8
    batched strided copies assemble the 6 lhsT tiles.
  - ScalarE evacuates PSUM with the bias add fused (bf16 out); output is
    stored in a packed [8-strip, 128, 14, 224] layout the host re-interleaves.
"""

import sys
from contextlib import ExitStack

for _p in ("/opt/trn_rl_repo",):
    if _p not in sys.path:
        sys.path.insert(0, _p)

import numpy as np

import concourse.bass as bass
import concourse.bacc as bacc
import concourse.mybir as mybir
from concourse.tile import TileContext

F32 = mybir.dt.float32
F32R = mybir.dt.float32r
BF16 = mybir.dt.bfloat16
AF = mybir.ActivationFunctionType

N_CORES = 8
O = 64
I = 64
KK = 3
D = KK * KK  # 9
ID = I * D  # 576
HH = 224
WW = 224
NP = HH // 2 + 1  # 113 pair slots
WPAD = WW + 4  # 228: 2 zero cols each side
GSTRIP = 8  # output strips per image
PPG = (HH // 2) // GSTRIP  # 14 output row-pairs per strip
QS = [0, 28, 56, 84]  # x quarter start slots (29 slots each, 1-slot overlap)
QLEN = 29


def build_nc(nb=4, n_w1=26, n_w1b=6, n_w2=7):
    """Build the per-core Bass program. nb: images per core."""
    nc = bacc.Bacc("TRN2", target_bir_lowering=False, debug=False)

    xp_t = nc.dram_tensor("xp", [nb, 128, NP, WPAD], BF16, kind="ExternalInput").ap()
    wl2_t = nc.dram_tensor("wl2", [O, 8 * ID], BF16, kind="ExternalInput").ap()
    eps3_t = nc.dram_tensor("eps3", [O, 8 * ID], BF16, kind="ExternalInput").ap()
    diag_t = nc.dram_tensor("diag", [O, ID], BF16, kind="ExternalInput").ap()
    epsw_t = nc.dram_tensor("epsw", [O, ID], BF16, kind="ExternalInput").ap()
    wloc_t = nc.dram_tensor("wloc", [O, ID], BF16, kind="ExternalInput").ap()
    ident_t = nc.dram_tensor("ident", [O, O], F32, kind="ExternalInput").ap()
    identb_t = nc.dram_tensor("identb", [O, O], BF16, kind="ExternalInput").ap()
    bias3_t = nc.dram_tensor("bias3", [3, O], F32, kind="ExternalInput").ap()
    out_t = nc.dram_tensor(
        "out", [nb, GSTRIP, 128, PPG, WW], BF16, kind="ExternalOutput"
    ).ap()

    with TileContext(nc) as tc, ExitStack() as stack:
        # ---------------- weight + bias sampling (one-time prologue) --------
        cp = stack.enter_context(tc.tile_pool(name="consts", bufs=1))
        wl2 = cp.tile([O, 8 * ID], BF16, name="wl2", tag="wl2")
        eps3 = cp.tile([O, 8 * ID], BF16, name="eps3", tag="eps3")
        diag = cp.tile([O, ID], BF16, name="diag", tag="diag")
        epsw = cp.tile([O, ID], BF16, name="epsw_s", tag="epsw_s")
        wloc = cp.tile([O, ID], BF16, name="wloc_s", tag="wloc_s")
        ident = cp.tile([O, O], F32, name="ident_s", tag="ident_s")
        identb = cp.tile([O, O], BF16, name="identb_s", tag="identb_s")
        b3 = cp.tile([O, 3], F32, name="b3", tag="b3")
        b3p = cp.tile([3, O], F32, name="b3p", tag="b3p")
        sp = cp.tile([O, ID], BF16, name="sp", tag="sp")
        tmp = cp.tile([O, ID], BF16, name="tmp", tag="tmp")
        prod = cp.tile([O, 8 * ID], BF16, name="prod", tag="prod")
        wsamp = cp.tile([O, ID], BF16, name="wsamp", tag="wsamp")
        # sampled weights duplicated side by side: the tap transposes read
        # free dim (q, i) -> both partition halves of the [128, .] transpose
        # destination in one PE pass (no partition-shift DMA afterwards)
        wsampd = cp.tile([O, 2 * ID], BF16, name="wsampd", tag="wsampd")
        bias = cp.tile([128, 1], F32, name="bias", tag="bias")
        sp_b = cp.tile([O, 1], F32, name="sp_b", tag="sp_b")
        # 6 lhsT tiles side by side: [128, 6*128] = A_s (s=0..2), B_s (3..5)
        wts = cp.tile([128, 6 * 128], BF16, name="wts", tag="wts")

        # sampling inputs split across both HWDGE rings so descriptor
        # generation pipelines; the two big sampling blocks lead the sync
        # ring ahead of the x quarters
        nc.sync.dma_start(wl2[:], wl2_t[:])
        nc.sync.dma_start(eps3[:], eps3_t[:])
        nc.sync.dma_start(epsw[:], epsw_t[:])
        nc.scalar.dma_start(diag[:], diag_t[:])
        nc.scalar.dma_start(identb[:], identb_t[:])
        nc.scalar.dma_start(ident[:], ident_t[:])
        nc.scalar.dma_start(b3p[:], bias3_t[:])
        nc.scalar.dma_start(wloc[:], wloc_t[:])

        # PE warm-up feed: zero tiles via VectorE (fast, no SWDGE latency).
        # Full 128-partition matmuls: 64-wide ones do NOT trip the HAM clock
        # gate (measured: 90x [64,256] warmup left the PE at 1.2 GHz).
        identr = cp.tile([128, 128], F32R, name="identr", tag="identr")
        junk = cp.tile([128, 448], F32R, name="junk", tag="junk")
        with tc.high_priority():
            nc.vector.memset(identr[:].bitcast(F32), 0.0)
            nc.vector.memset(junk[:].bitcast(F32), 0.0)
        # zero the dead lhsT quadrants (A: q1/p1, B: q0/p0) in one shot
        nc.gpsimd.memset(wts[:].bitcast(F32), 0.0)

        with tc.tile_pool(name="prol", bufs=1, space="PSUM") as wp:
            # HAM needs ~3.4us of sustained full-width matmul activity to
            # lift the PE 1.2 -> 2.4 GHz; these also bridge PE-idle windows
            # while VectorE/ScalarE run the sampling chain.
            warm = wp.tile([128, 448], F32, name="warm")
            for k in range(n_w1):
                nc.tensor.matmul(
                    warm[:], identr[:], junk[:], start=(k == 0), stop=(k == n_w1 - 1)
                )

            # bias3 arrives as [3, 64]; transpose to [64, 3] on the PE (a
            # partition-major DMA of 64x3 elements costs ~17us in descriptors)
            bp_ps = wp.tile([O, 3], F32, name="bp_ps")
            nc.tensor.matmul(bp_ps[:], b3p[:], ident[0:3, 0:3], start=True, stop=True)

            for k in range(n_w1b):
                nc.tensor.matmul(
                    warm[:], identr[:], junk[:], start=(k == 0), stop=(k == n_w1b - 1)
                )

            # ---- VectorE sampling chain (all contiguous bf16) ------------
            # prod[o,(b,i,a)] = wl2 * eps3 elementwise; wl2 is host-masked
            # to the strict-lower taps and eps3 is eps_w host-broadcast over
            # a, so the b-tree-sum IS (tril(L,-1) @ eps).
            nc.vector.tensor_mul(prod[:], wl2[:], eps3[:])
            nc.vector.tensor_add(prod[:, 0 : 4 * ID], prod[:, 0 : 4 * ID],
                                 prod[:, 4 * ID : 8 * ID])
            nc.vector.tensor_add(prod[:, 0 : 2 * ID], prod[:, 0 : 2 * ID],
                                 prod[:, 2 * ID : 4 * ID])
            nc.vector.tensor_add(prod[:, 0:ID], prod[:, 0:ID], prod[:, ID : 2 * ID])
            nc.vector.tensor_copy(b3[:], bp_ps[:])

            # softplus(diag) on ScalarE: Exp then Ln (ln(e^x + 1)); there is
            # no Softplus LUT in this toolchain.
            nc.scalar.activation(sp[:], diag[:], AF.Exp)
            nc.scalar.activation(sp[:], sp[:], AF.Ln, bias=1.0)

            # wsamp = wloc + softplus(diag)*eps + strict_lower (twice, for
            # the both-halves transpose trick)
            nc.vector.tensor_mul(tmp[:], sp[:], epsw[:])
            nc.vector.tensor_add(wsamp[:], wloc[:], tmp[:])
            nc.vector.tensor_add(wsampd[:, 0:ID], wsamp[:], prod[:, 0:ID])
            nc.vector.tensor_add(wsampd[:, ID : 2 * ID], wsamp[:], prod[:, 0:ID])

            # ---- tap transposes + lhsT assembly --------------------------
            # T[t][ich,och] = wsamp[och, ich*9+t], written to BOTH partition
            # halves of ptA/ptB at once via the duplicated wsampd free dim.
            ptA = wp.tile([128, 5 * O], BF16, name="ptA")
            ptB = wp.tile([128, 4 * O], BF16, name="ptB")
            for a in range(D):
                w_a2 = bass.AP(
                    tensor=wsampd[:].tensor,
                    offset=wsampd[:].offset + a,
                    ap=[list(p) for p in wsampd[:].ap[:1]] + [[ID, 2], [D, I]],
                )
                dst_pt = ptA if a < 5 else ptB
                c = a if a < 5 else a - 5
                nc.tensor.matmul(
                    dst_pt[:, c * O : (c + 1) * O],
                    w_a2,
                    identb[:],
                    is_transpose=True,
                    start=(c == 0),
                    stop=(c == (4 if a < 5 else 3)),
                    skip_group_check=True,
                )

            # keep the PE busy while the lhsT copies run
            for k in range(n_w2):
                nc.tensor.matmul(
                    warm[:], identr[:], junk[:], start=(k == 0), stop=(k == n_w2 - 1)
                )

            # batched strided copies (dst stride 128, src stride 64):
            #   A_s: [q0,p0]=T[3+s]  [q0,p1]=T[s]  [q1,p0]=T[s]  [q1,p1]=0
            #   B_s: [q0,p0]=0  [q0,p1]=T[6+s]  [q1,p0]=T[6+s]  [q1,p1]=T[3+s]
            def bcopy(eng, dst_c0, dst_n, src_pt, src_half, src_c0):
                pstr = wts[:].ap[0][0]
                dst = bass.AP(
                    tensor=wts[:].tensor,
                    offset=wts[:].offset + src_half * 64 * pstr + dst_c0,
                    ap=[[pstr, 64], [128, dst_n], [1, O]],
                )
                s_ = src_pt[src_half * 64 : src_half * 64 + 64,
                            src_c0 : src_c0 + dst_n * O]
                src = bass.AP(
                    tensor=s_.tensor, offset=s_.offset,
                    ap=[list(s_.ap[0])] + [[O, dst_n], [1, O]],
                )
                if eng == "v":
                    nc.vector.tensor_copy(dst, src)
                else:
                    nc.scalar.activation(dst, src, AF.Copy)

            bcopy("v", O, 3, ptA, 0, 0)        # A q0,p1 <- T[0..2]
            bcopy("s", 0, 3, ptA, 1, 0)        # A q1,p0 <- T[0..2]
            bcopy("v", 0, 2, ptA, 0, 3 * O)    # A q0,p0 <- T[3..4]
            bcopy("s", 2 * 128, 1, ptB, 0, 0)  # A2 q0,p0 <- T[5]
            bcopy("v", 3 * 128 + O, 3, ptB, 0, O)      # B q0,p1 <- T[6..8]
            bcopy("s", 3 * 128, 3, ptB, 1, O)          # B q1,p0 <- T[6..8]
            bcopy("v", 3 * 128 + O, 2, ptA, 1, 3 * O)  # B0-1 q1,p1 <- T[3..4]
            bcopy("s", 5 * 128 + O, 1, ptB, 1, 0)      # B2 q1,p1 <- T[5]

            # bias = bias_loc + eps_b * softplus(bias_ro)  (off critical path)
            nc.scalar.activation(sp_b[:], b3[:, 1:2], AF.Exp)
            nc.scalar.activation(sp_b[:], sp_b[:], AF.Ln, bias=1.0)
            nc.vector.tensor_mul(sp_b[:], sp_b[:], b3[:, 2:3])
            nc.vector.tensor_add(bias[0:O, :], b3[:, 0:1], sp_b[:])
            nc.scalar.dma_start(bias[O:128, :], bias[0:O, :])

        # ---------------- convolution ---------------------------------------
        # per psum tile t (out rows 4t..4t+3 of one image):
        #   acc[(p,och), (k in {2t,2t+1}, c)] = out row 2k+p
        #   A_s: rhs slots (2t, 2t+1)   B_s: rhs slots (2t+1, 2t+2)
        #   rhs col start = s+1 (packed col cc = image col + 2)
        xqp = stack.enter_context(tc.tile_pool(name="xq", bufs=6))
        op = stack.enter_context(tc.tile_pool(name="ostrip", bufs=2))
        pp = stack.enter_context(tc.tile_pool(name="acc", bufs=8, space="PSUM"))
        for n in range(nb):
            xq = []
            for q in range(4):
                xt = xqp.tile([128, QLEN, WPAD], BF16, name="xq")
                nc.sync.dma_start(xt[:], xp_t[n, :, QS[q] : QS[q] + QLEN, :])
                xq.append(xt)
            for g in range(GSTRIP):
                os_ = op.tile([128, PPG, WW], BF16, name="os_")
                last = n == nb - 1 and g == GSTRIP - 1
                for tt in range(PPG // 2):
                    t = (PPG // 2) * g + tt
                    qi = (t >= 14) + (t >= 28) + (t >= 42)
                    lo = 2 * t - QS[qi]
                    xs = xq[qi]
                    acc = pp.tile([128, 2, WW], F32, name="acc")
                    for s in range(3):
                        rhs_a = xs[:, lo : lo + 2, s + 1 : s + 1 + WW]
                        rhs_b = xs[:, lo + 1 : lo + 3, s + 1 : s + 1 + WW]
                        nc.tensor.matmul(
                            acc[:],
                            wts[:, s * 128 : (s + 1) * 128],
                            rhs_a,
                            start=(s == 0),
                            stop=False,
                        )
                        nc.tensor.matmul(
                            acc[:],
                            wts[:, (3 + s) * 128 : (4 + s) * 128],
                            rhs_b,
                            start=False,
                            stop=(s == 2),
                        )
                    nc.scalar.activation(
                        os_[:, 2 * tt : 2 * tt + 2, :],
                        acc[:],
                        AF.Identity,
                        bias=bias[:, 0:1],
                    )
                    # taper: stream the final strip out in pieces so the
                    # kernel does not end on a full-size store
                    if last and tt in (1, 3, 5):
                        k0, k1 = {1: (0, 4), 3: (4, 8), 5: (8, 12)}[tt]
                        nc.scalar.dma_start(
                            out_t[n, g, :, k0:k1, :], os_[:, k0:k1, :]
                        )
                if last:
                    nc.scalar.dma_start(out_t[n, g, :, 12:PPG, :], os_[:, 12:PPG, :])
                else:
                    nc.scalar.dma_start(out_t[n, g], os_[:])

    nc.compile()
    return nc


_CACHED_NC = None


def _pack_x(x_shard_bf):
    """[nb, 64, 224, 224] bf16 -> [nb, 128, 113, 228] staggered parity pack."""
    nb = x_shard_bf.shape[0]
    xp = np.zeros((nb, 128, NP, WPAD), dtype=x_shard_bf.dtype)
    xp[:, 0:64, 0 : HH // 2, 2 : WW + 2] = x_shard_bf[:, :, 0::2, :]
    xp[:, 64:128, 1 : HH // 2 + 1, 2 : WW + 2] = x_shard_bf[:, :, 1::2, :]
    return xp


def _host_inputs(x_shard, weight_loc, weight_L, bias_loc, bias_ro, eps_w, eps_b):
    import ml_dtypes

    bf = ml_dtypes.bfloat16
    wlf = np.asarray(weight_L, np.float32)  # [O, I, D(a), D(b)]
    mask = np.tril(np.ones((D, D), np.float32), -1)  # [a, b]: a > b
    wl2 = (wlf * mask).transpose(0, 3, 1, 2)[:, 0:8]  # [O, b, I, a]
    dg = np.diagonal(wlf, axis1=2, axis2=3)  # [O, I, D]
    ew = np.asarray(eps_w, np.float32)  # [O, I, D(b)]
    eps3 = np.broadcast_to(
        ew.transpose(0, 2, 1)[:, 0:8, :, None], (O, 8, I, D)
    )  # [O, b, I, a]: eps_w[o,i,b] for every a
    return {
        "xp": _pack_x(np.asarray(x_shard).astype(bf)),
        "wl2": np.ascontiguousarray(wl2.reshape(O, 8 * ID)).astype(bf),
        "eps3": np.ascontiguousarray(eps3.reshape(O, 8 * ID)).astype(bf),
        "diag": np.ascontiguousarray(dg.reshape(O, ID)).astype(bf),
        "epsw": np.ascontiguousarray(ew.reshape(O, ID)).astype(bf),
        "wloc": np.ascontiguousarray(
            np.asarray(weight_loc, np.float32).reshape(O, ID)
        ).astype(bf),
        "ident": np.eye(O, dtype=np.float32),
        "identb": np.eye(O, dtype=np.float32).astype(bf),
        "bias3": np.ascontiguousarray(
            np.stack([bias_loc, bias_ro, eps_b]).astype(np.float32)
        ),
    }


def kernel(x, weight_loc, weight_L, bias_loc, bias_ro, eps_w, eps_b):
    global _CACHED_NC
    from concourse.bass_utils import run_bass_kernel_spmd

    x = np.asarray(x, np.float32)
    nb = x.shape[0] // N_CORES
    if _CACHED_NC is None:
        _CACHED_NC = build_nc(nb=nb)
    nc = _CACHED_NC

    import ml_dtypes

    x_bf = x.astype(ml_dtypes.bfloat16)
    in_maps = [
        _host_inputs(
            x_bf[c * nb : (c + 1) * nb],
            np.asarray(weight_loc),
            np.asarray(weight_L),
            np.asarray(bias_loc),
            np.asarray(bias_ro),
            np.asarray(eps_w),
            np.asarray(eps_b),
        )
        for c in range(N_CORES)
    ]
    res = run_bass_kernel_spmd(nc, in_maps, list(range(N_CORES)))
    outs = []
    for c in range(N_CORES):
        o = np.asarray(res.results[c]["out"])  # [nb, 8, 128, 14, 224] bf16
        o = o.reshape(nb, GSTRIP, 2, O, PPG, WW).transpose(0, 3, 1, 4, 2, 5)
        outs.append(o.reshape(nb, O, HH, WW).astype(np.float32))
    return np.concatenate(outs, axis=0)


# revision 18
# speedup vs baseline: 1.6306x; 1.0128x over previous
"""Trainium2 Bass kernel for nn_Conv2DExperimental (MVN-sampled 3x3 conv).

Computation (per the nn.Module):
  L    = tril(weight_L, -1) + softplus(diag(weight_L)) * I      # [O,I,D,D], D=9
  w    = weight_loc + einsum('oiab,oib->oia', L, eps_w)         # [O,I,3,3]
  b    = bias_loc + eps_b * softplus(bias_ro)                   # [O]
  out  = conv2d(x, w, SAME, NCHW) + b
  with O = I = 64 channels, x [32, 64, 224, 224].

Distribution: data-parallel over the batch dim of x (32 images -> 8 cores x 4),
with the weight sampling replicated on every core (it is tiny).

Per-core kernel (row-parity conv, 75% PE utilization):
  - x is host-packed bf16 into SBUF layout [128, 113, 228]: partitions
    (parity q, in-channel), where q=0 slot k holds image row 2k and q=1 slot k
    holds row 2k-1 (staggered), columns padded by 2 on both sides.  Halo
    rows/columns are pre-zeroed on the host, so the kernel needs no memsets.
    Images stream in as 29-slot quarters through a 6-deep tile pool, so the
    HBM reads are consumption-paced instead of bursting against the output
    stores (HBM is ~358 GB/s per core).
  - output psum tiles are [128 = (row-parity p, out-channel), 2 pairs x 224]:
    out row 2k+p.  Per tile, 6 matmuls (2 input row-groups x 3 column shifts)
    apply all 9 taps exactly once per output: lhsT tiles have 3 of 4
    64x64 quadrants live (vs 2 of 4 for the image-paired block-diagonal
    scheme) -> 1.5x less PE time.
  - sampling: the host pre-layouts weight_L as a masked strict-lower
    [O, (b, i, a)] block plus the diagonal [O, (i, a)], and pre-broadcasts
    eps_w to the same shape (pure data reshuffle), so L @ eps is one
    contiguous VectorE multiply + 3 tree adds, and softplus(diag) is a
    contiguous ScalarE op.  The 9 tap matrices are PE-transposed into both
    partition halves at once (weights duplicated side by side), then 8
    batched strided copies assemble the 6 lhsT tiles.
  - ScalarE evacuates PSUM with the bias add fused (bf16 out); output is
    stored in a packed [8-strip, 128, 14, 224] layout the host re-interleaves.
"""

import sys
from contextlib import ExitStack

for _p in ("/opt/trn_rl_repo",):
    if _p not in sys.path:
        sys.path.insert(0, _p)

import numpy as np

import concourse.bass as bass
import concourse.bacc as bacc
import concourse.mybir as mybir
from concourse.tile import TileContext

F32 = mybir.dt.float32
F32R = mybir.dt.float32r
BF16 = mybir.dt.bfloat16
AF = mybir.ActivationFunctionType

N_CORES = 8
O = 64
I = 64
KK = 3
D = KK * KK  # 9
ID = I * D  # 576
HH = 224
WW = 224
NP = HH // 2 + 1  # 113 pair slots
WPAD = WW + 4  # 228: 2 zero cols each side
GSTRIP = 8  # output strips per image
PPG = (HH // 2) // GSTRIP  # 14 output row-pairs per strip
QS = [0, 28, 56, 84]  # x quarter start slots (29 slots each, 1-slot overlap)
QLEN = 29


def build_nc(nb=4, n_w1=26, n_w1b=6, n_w2=11):
    """Build the per-core Bass program. nb: images per core."""
    nc = bacc.Bacc("TRN2", target_bir_lowering=False, debug=False)

    xp_t = nc.dram_tensor("xp", [nb, 128, NP, WPAD], BF16, kind="ExternalInput").ap()
    wl2_t = nc.dram_tensor("wl2", [O, 8 * ID], BF16, kind="ExternalInput").ap()
    eps3_t = nc.dram_tensor("eps3", [O, 8 * ID], BF16, kind="ExternalInput").ap()
    diag_t = nc.dram_tensor("diag", [O, ID], BF16, kind="ExternalInput").ap()
    epsw_t = nc.dram_tensor("epsw", [O, ID], BF16, kind="ExternalInput").ap()
    wloc_t = nc.dram_tensor("wloc", [O, ID], BF16, kind="ExternalInput").ap()
    ident_t = nc.dram_tensor("ident", [O, O], F32, kind="ExternalInput").ap()
    identb_t = nc.dram_tensor("identb", [O, O], BF16, kind="ExternalInput").ap()
    bias3_t = nc.dram_tensor("bias3", [3, O], F32, kind="ExternalInput").ap()
    out_t = nc.dram_tensor(
        "out", [nb, GSTRIP, 128, PPG, WW], BF16, kind="ExternalOutput"
    ).ap()

    with TileContext(nc) as tc, ExitStack() as stack:
        # ---------------- weight + bias sampling (one-time prologue) --------
        cp = stack.enter_context(tc.tile_pool(name="consts", bufs=1))
        wl2 = cp.tile([O, 8 * ID], BF16, name="wl2", tag="wl2")
        eps3 = cp.tile([O, 8 * ID], BF16, name="eps3", tag="eps3")
        diag = cp.tile([O, ID], BF16, name="diag", tag="diag")
        epsw = cp.tile([O, ID], BF16, name="epsw_s", tag="epsw_s")
        wloc = cp.tile([O, ID], BF16, name="wloc_s", tag="wloc_s")
        ident = cp.tile([O, O], F32, name="ident_s", tag="ident_s")
        identb = cp.tile([O, O], BF16, name="identb_s", tag="identb_s")
        b3 = cp.tile([O, 3], F32, name="b3", tag="b3")
        b3p = cp.tile([3, O], F32, name="b3p", tag="b3p")
        sp = cp.tile([O, ID], BF16, name="sp", tag="sp")
        tmp = cp.tile([O, ID], BF16, name="tmp", tag="tmp")
        prod = cp.tile([O, 8 * ID], BF16, name="prod", tag="prod")
        wsamp = cp.tile([O, ID], BF16, name="wsamp", tag="wsamp")
        # sampled weights duplicated side by side: the tap transposes read
        # free dim (q, i) -> both partition halves of the [128, .] transpose
        # destination in one PE pass (no partition-shift DMA afterwards)
        wsampd = cp.tile([O, 2 * ID], BF16, name="wsampd", tag="wsampd")
        bias = cp.tile([128, 1], F32, name="bias", tag="bias")
        sp_b = cp.tile([O, 1], F32, name="sp_b", tag="sp_b")
        # 6 lhsT tiles side by side: [128, 6*128] = A_s (s=0..2), B_s (3..5)
        wts = cp.tile([128, 6 * 128], BF16, name="wts", tag="wts")

        # sampling inputs split across both HWDGE rings so the two big
        # blocks (wl2, eps3) transfer in parallel ahead of everything else;
        # the x quarters queue behind them on the sync ring
        nc.sync.dma_start(wl2[:], wl2_t[:])
        nc.sync.dma_start(epsw[:], epsw_t[:])
        nc.sync.dma_start(ident[:], ident_t[:])
        nc.sync.dma_start(b3p[:], bias3_t[:])
        nc.scalar.dma_start(diag[:], diag_t[:])
        nc.scalar.dma_start(eps3[:], eps3_t[:])
        nc.scalar.dma_start(identb[:], identb_t[:])
        nc.scalar.dma_start(wloc[:], wloc_t[:])

        # PE warm-up feed: zero tiles via VectorE (fast, no SWDGE latency).
        # Full 128-partition matmuls: 64-wide ones do NOT trip the HAM clock
        # gate (measured: 90x [64,256] warmup left the PE at 1.2 GHz).
        identr = cp.tile([128, 128], F32R, name="identr", tag="identr")
        junk = cp.tile([128, 448], F32R, name="junk", tag="junk")
        with tc.high_priority():
            nc.vector.memset(identr[:].bitcast(F32), 0.0)
            nc.vector.memset(junk[:].bitcast(F32), 0.0)
        # zero the dead lhsT quadrants (A: q1/p1, B: q0/p0) in one shot
        nc.gpsimd.memset(wts[:].bitcast(F32), 0.0)

        with tc.tile_pool(name="prol", bufs=1, space="PSUM") as wp:
            # HAM needs ~3.4us of sustained full-width matmul activity to
            # lift the PE 1.2 -> 2.4 GHz; these also bridge PE-idle windows
            # while VectorE/ScalarE run the sampling chain.
            warm = wp.tile([128, 448], F32, name="warm")
            for k in range(n_w1):
                nc.tensor.matmul(
                    warm[:], identr[:], junk[:], start=(k == 0), stop=(k == n_w1 - 1)
                )

            # bias3 arrives as [3, 64]; transpose to [64, 3] on the PE (a
            # partition-major DMA of 64x3 elements costs ~17us in descriptors)
            bp_ps = wp.tile([O, 3], F32, name="bp_ps")
            nc.tensor.matmul(bp_ps[:], b3p[:], ident[0:3, 0:3], start=True, stop=True)

            for k in range(n_w1b):
                nc.tensor.matmul(
                    warm[:], identr[:], junk[:], start=(k == 0), stop=(k == n_w1b - 1)
                )

            # ---- VectorE sampling chain (all contiguous bf16) ------------
            # prod[o,(b,i,a)] = wl2 * eps3 elementwise; wl2 is host-masked
            # to the strict-lower taps and eps3 is eps_w host-broadcast over
            # a, so the b-tree-sum IS (tril(L,-1) @ eps).
            nc.vector.tensor_mul(prod[:], wl2[:], eps3[:])
            nc.vector.tensor_add(prod[:, 0 : 4 * ID], prod[:, 0 : 4 * ID],
                                 prod[:, 4 * ID : 8 * ID])
            nc.vector.tensor_add(prod[:, 0 : 2 * ID], prod[:, 0 : 2 * ID],
                                 prod[:, 2 * ID : 4 * ID])
            nc.vector.tensor_add(prod[:, 0:ID], prod[:, 0:ID], prod[:, ID : 2 * ID])
            nc.vector.tensor_copy(b3[:], bp_ps[:])

            # softplus(diag) on ScalarE: Exp then Ln (ln(e^x + 1)); there is
            # no Softplus LUT in this toolchain.
            nc.scalar.activation(sp[:], diag[:], AF.Exp)
            nc.scalar.activation(sp[:], sp[:], AF.Ln, bias=1.0)

            # wsamp = wloc + softplus(diag)*eps + strict_lower (twice, for
            # the both-halves transpose trick)
            nc.vector.tensor_mul(tmp[:], sp[:], epsw[:])
            nc.vector.tensor_add(wsamp[:], wloc[:], tmp[:])
            nc.vector.tensor_add(wsampd[:, 0:ID], wsamp[:], prod[:, 0:ID])
            nc.vector.tensor_add(wsampd[:, ID : 2 * ID], wsamp[:], prod[:, 0:ID])

            # ---- tap transposes + lhsT assembly --------------------------
            # T[t][ich,och] = wsamp[och, ich*9+t], written to BOTH partition
            # halves of ptA/ptB at once via the duplicated wsampd free dim.
            ptA = wp.tile([128, 5 * O], BF16, name="ptA")
            ptB = wp.tile([128, 4 * O], BF16, name="ptB")
            for a in range(D):
                w_a2 = bass.AP(
                    tensor=wsampd[:].tensor,
                    offset=wsampd[:].offset + a,
                    ap=[list(p) for p in wsampd[:].ap[:1]] + [[ID, 2], [D, I]],
                )
                dst_pt = ptA if a < 5 else ptB
                c = a if a < 5 else a - 5
                nc.tensor.matmul(
                    dst_pt[:, c * O : (c + 1) * O],
                    w_a2,
                    identb[:],
                    is_transpose=True,
                    start=(c == 0),
                    stop=(c == (4 if a < 5 else 3)),
                    skip_group_check=True,
                )

            # keep the PE busy while the lhsT copies run
            for k in range(n_w2):
                nc.tensor.matmul(
                    warm[:], identr[:], junk[:], start=(k == 0), stop=(k == n_w2 - 1)
                )

            # batched strided copies (dst stride 128, src stride 64):
            #   A_s: [q0,p0]=T[3+s]  [q0,p1]=T[s]  [q1,p0]=T[s]  [q1,p1]=0
            #   B_s: [q0,p0]=0  [q0,p1]=T[6+s]  [q1,p0]=T[6+s]  [q1,p1]=T[3+s]
            def bcopy(eng, dst_c0, dst_n, src_pt, src_half, src_c0):
                pstr = wts[:].ap[0][0]
                dst = bass.AP(
                    tensor=wts[:].tensor,
                    offset=wts[:].offset + src_half * 64 * pstr + dst_c0,
                    ap=[[pstr, 64], [128, dst_n], [1, O]],
                )
                s_ = src_pt[src_half * 64 : src_half * 64 + 64,
                            src_c0 : src_c0 + dst_n * O]
                src = bass.AP(
                    tensor=s_.tensor, offset=s_.offset,
                    ap=[list(s_.ap[0])] + [[O, dst_n], [1, O]],
                )
                if eng == "v":
                    nc.vector.tensor_copy(dst, src)
                else:
                    nc.scalar.activation(dst, src, AF.Copy)

            bcopy("v", O, 3, ptA, 0, 0)        # A q0,p1 <- T[0..2]
            bcopy("s", 0, 3, ptA, 1, 0)        # A q1,p0 <- T[0..2]
            bcopy("v", 0, 2, ptA, 0, 3 * O)    # A q0,p0 <- T[3..4]
            bcopy("s", 2 * 128, 1, ptB, 0, 0)  # A2 q0,p0 <- T[5]
            bcopy("v", 3 * 128 + O, 3, ptB, 0, O)      # B q0,p1 <- T[6..8]
            bcopy("s", 3 * 128, 3, ptB, 1, O)          # B q1,p0 <- T[6..8]
            bcopy("v", 3 * 128 + O, 2, ptA, 1, 3 * O)  # B0-1 q1,p1 <- T[3..4]
            bcopy("s", 5 * 128 + O, 1, ptB, 1, 0)      # B2 q1,p1 <- T[5]

            # bias = bias_loc + eps_b * softplus(bias_ro)  (off critical path)
            nc.scalar.activation(sp_b[:], b3[:, 1:2], AF.Exp)
            nc.scalar.activation(sp_b[:], sp_b[:], AF.Ln, bias=1.0)
            nc.vector.tensor_mul(sp_b[:], sp_b[:], b3[:, 2:3])
            nc.vector.tensor_add(bias[0:O, :], b3[:, 0:1], sp_b[:])
            nc.scalar.dma_start(bias[O:128, :], bias[0:O, :])

        # ---------------- convolution ---------------------------------------
        # per psum tile t (out rows 4t..4t+3 of one image):
        #   acc[(p,och), (k in {2t,2t+1}, c)] = out row 2k+p
        #   A_s: rhs slots (2t, 2t+1)   B_s: rhs slots (2t+1, 2t+2)
        #   rhs col start = s+1 (packed col cc = image col + 2)
        # bufs=2: only two quarters in flight, so image reads are paced by
        # conv consumption instead of queueing 6.4MB against the stores and
        # the prologue loads (whole-packet round-robin on the SDMA engines)
        xqp = stack.enter_context(tc.tile_pool(name="xq", bufs=2))
        op = stack.enter_context(tc.tile_pool(name="ostrip", bufs=2))
        pp = stack.enter_context(tc.tile_pool(name="acc", bufs=8, space="PSUM"))
        for n in range(nb):
            xq = []
            for q in range(4):
                xt = xqp.tile([128, QLEN, WPAD], BF16, name="xq")
                nc.sync.dma_start(xt[:], xp_t[n, :, QS[q] : QS[q] + QLEN, :])
                xq.append(xt)
            for g in range(GSTRIP):
                os_ = op.tile([128, PPG, WW], BF16, name="os_")
                last = n == nb - 1 and g == GSTRIP - 1
                for tt in range(PPG // 2):
                    t = (PPG // 2) * g + tt
                    qi = (t >= 14) + (t >= 28) + (t >= 42)
                    lo = 2 * t - QS[qi]
                    xs = xq[qi]
                    acc = pp.tile([128, 2, WW], F32, name="acc")
                    for s in range(3):
                        rhs_a = xs[:, lo : lo + 2, s + 1 : s + 1 + WW]
                        rhs_b = xs[:, lo + 1 : lo + 3, s + 1 : s + 1 + WW]
                        nc.tensor.matmul(
                            acc[:],
                            wts[:, s * 128 : (s + 1) * 128],
                            rhs_a,
                            start=(s == 0),
                            stop=False,
                        )
                        nc.tensor.matmul(
                            acc[:],
                            wts[:, (3 + s) * 128 : (4 + s) * 128],
                            rhs_b,
                            start=False,
                            stop=(s == 2),
                        )
                    nc.scalar.activation(
                        os_[:, 2 * tt : 2 * tt + 2, :],
                        acc[:],
                        AF.Identity,
                        bias=bias[:, 0:1],
                    )
                    # taper: stream the final strip out in pieces so the
                    # kernel does not end on a full-size store
                    if last and tt in (1, 3, 5):
                        k0, k1 = {1: (0, 4), 3: (4, 8), 5: (8, 12)}[tt]
                        nc.scalar.dma_start(
                            out_t[n, g, :, k0:k1, :], os_[:, k0:k1, :]
                        )
                if last:
                    nc.scalar.dma_start(out_t[n, g, :, 12:PPG, :], os_[:, 12:PPG, :])
                else:
                    nc.scalar.dma_start(out_t[n, g], os_[:])

    nc.compile()
    return nc


_CACHED_NC = None


def _pack_x(x_shard_bf):
    """[nb, 64, 224, 224] bf16 -> [nb, 128, 113, 228] staggered parity pack."""
    nb = x_shard_bf.shape[0]
    xp = np.zeros((nb, 128, NP, WPAD), dtype=x_shard_bf.dtype)
    xp[:, 0:64, 0 : HH // 2, 2 : WW + 2] = x_shard_bf[:, :, 0::2, :]
    xp[:, 64:128, 1 : HH // 2 + 1, 2 : WW + 2] = x_shard_bf[:, :, 1::2, :]
    return xp


def _host_inputs(x_shard, weight_loc, weight_L, bias_loc, bias_ro, eps_w, eps_b):
    import ml_dtypes

    bf = ml_dtypes.bfloat16
    wlf = np.asarray(weight_L, np.float32)  # [O, I, D(a), D(b)]
    mask = np.tril(np.ones((D, D), np.float32), -1)  # [a, b]: a > b
    wl2 = (wlf * mask).transpose(0, 3, 1, 2)[:, 0:8]  # [O, b, I, a]
    dg = np.diagonal(wlf, axis1=2, axis2=3)  # [O, I, D]
    ew = np.asarray(eps_w, np.float32)  # [O, I, D(b)]
    eps3 = np.broadcast_to(
        ew.transpose(0, 2, 1)[:, 0:8, :, None], (O, 8, I, D)
    )  # [O, b, I, a]: eps_w[o,i,b] for every a
    return {
        "xp": _pack_x(np.asarray(x_shard).astype(bf)),
        "wl2": np.ascontiguousarray(wl2.reshape(O, 8 * ID)).astype(bf),
        "eps3": np.ascontiguousarray(eps3.reshape(O, 8 * ID)).astype(bf),
        "diag": np.ascontiguousarray(dg.reshape(O, ID)).astype(bf),
        "epsw": np.ascontiguousarray(ew.reshape(O, ID)).astype(bf),
        "wloc": np.ascontiguousarray(
            np.asarray(weight_loc, np.float32).reshape(O, ID)
        ).astype(bf),
        "ident": np.eye(O, dtype=np.float32),
        "identb": np.eye(O, dtype=np.float32).astype(bf),
        "bias3": np.ascontiguousarray(
            np.stack([bias_loc, bias_ro, eps_b]).astype(np.float32)
        ),
    }


def kernel(x, weight_loc, weight_L, bias_loc, bias_ro, eps_w, eps_b):
    global _CACHED_NC
    from concourse.bass_utils import run_bass_kernel_spmd

    x = np.asarray(x, np.float32)
    nb = x.shape[0] // N_CORES
    if _CACHED_NC is None:
        _CACHED_NC = build_nc(nb=nb)
    nc = _CACHED_NC

    import ml_dtypes

    x_bf = x.astype(ml_dtypes.bfloat16)
    in_maps = [
        _host_inputs(
            x_bf[c * nb : (c + 1) * nb],
            np.asarray(weight_loc),
            np.asarray(weight_L),
            np.asarray(bias_loc),
            np.asarray(bias_ro),
            np.asarray(eps_w),
            np.asarray(eps_b),
        )
        for c in range(N_CORES)
    ]
    res = run_bass_kernel_spmd(nc, in_maps, list(range(N_CORES)))
    outs = []
    for c in range(N_CORES):
        o = np.asarray(res.results[c]["out"])  # [nb, 8, 128, 14, 224] bf16
        o = o.reshape(nb, GSTRIP, 2, O, PPG, WW).transpose(0, 3, 1, 4, 2, 5)
        outs.append(o.reshape(nb, O, HH, WW).astype(np.float32))
    return np.concatenate(outs, axis=0)
